# revision 17
# baseline (speedup 1.0000x reference)
"""Trainium2 Bass kernel for AttentionalPlanarRemapping.

out[n,c,h,w] = sum_d softmax(atts[n,c,:])[d] * images[n,d,h,w]

Per-sample: W = softmax(atts[n]) [C,C]; out[n] = W @ images[n].reshape(C, H*W).

Sharding: data-parallel over N across 8 cores (4 samples per core).

Host preprocessing inside kernel(): atts is passed TRANSPOSED per sample
(attsT[n] = atts[n].T, layout [d, c]) so no on-device transposition of the
512x512 weight matrix is needed: attsT loads with the contraction dim d on
partitions, which is exactly the matmul lhsT layout.

Per-core plan (sample-by-sample, pipelined across samples by Tile pools):
  1. DMA attsT[n] (2 halves) -> A [128, 2(kd), 512(c)]  (d on partitions)
  2. DMA images[n] (2 halves) -> X [128, 2(kd), 1024]   (d on partitions)
  3. E = exp(A) f32r elementwise (ACT; no max-sub: |atts| < 6 so exp is safe)
  4. R[p,c] = sum_d E[d,c] replicated across partitions via ones.T @ E (PE)
  5. E[:, c] *= 1/R[p, c]  (DVE reciprocal + 4 elementwise muls)
  6. matmul f32r full-rate: psum[c128, hw1024] += E[:,kc-blk].T @ X (8 MM/kc)
  7. evict psum -> O (plain copy), alternating ACT/DVE
  8. DMA O -> out[n] (2 halves, SWDGE so stores don't block the load queue)
"""

import numpy as np
from contextlib import ExitStack

import concourse.bass as bass
import concourse.mybir as mybir
import concourse.tile as tile
from concourse import bacc
from concourse.bass_utils import run_bass_kernel_spmd

N, C, H, W = 32, 512, 32, 32
HW = H * W                      # 1024
NCORES = 8
NPC = N // NCORES               # 4 samples per core
P = 128
KC = C // P                     # 4 chunks over output channel c
KD = C // P                     # 4 chunks over contraction d
NT = 512                        # matmul moving free dim (one PSUM bank of f32)
NHT = HW // NT                  # 2

F32 = mybir.dt.float32
F32R = mybir.dt.float32r
AF = mybir.ActivationFunctionType
AX = mybir.AxisListType


def build_nc():
    nc = bacc.Bacc("TRN2", target_bir_lowering=False, debug=False)

    images = nc.dram_tensor("images", [NPC, C, HW], F32R, kind="ExternalInput").ap()
    attsT = nc.dram_tensor("attsT", [NPC, C, C], F32, kind="ExternalInput").ap()
    out = nc.dram_tensor("out", [NPC, C, HW], F32, kind="ExternalOutput").ap()

    with ExitStack() as ctx:
        tc = ctx.enter_context(tile.TileContext(nc))

        const_pool = ctx.enter_context(tc.tile_pool(name="const", bufs=1))
        ones_f32 = const_pool.tile([P, P], F32)
        ones = const_pool.tile([P, P], F32R)

        a_pool = ctx.enter_context(tc.tile_pool(name="a", bufs=2))
        e_pool = ctx.enter_context(tc.tile_pool(name="e", bufs=2))
        x_pool = ctx.enter_context(tc.tile_pool(name="x", bufs=3))
        o_pool = ctx.enter_context(tc.tile_pool(name="o", bufs=2))
        st_pool = ctx.enter_context(tc.tile_pool(name="st", bufs=2))
        sm_psum = ctx.enter_context(tc.tile_pool(name="smp", bufs=2, space="PSUM"))
        mm_psum = ctx.enter_context(tc.tile_pool(name="mmp", bufs=2, space="PSUM"))

        first = True
        for n in range(NPC):
            # ---- input DMAs first (halves so consumers start earlier) ----
            a_h = []
            for h in range(2):
                a_t = a_pool.tile([P, 2, C], F32, name=f"a{n}_{h}", tag=f"a{h}")
                nc.sync.dma_start(
                    a_t[:],
                    attsT[n][h * 256 : (h + 1) * 256].rearrange(
                        "(kd p) c -> p kd c", p=P
                    ),
                )
                a_h.append(a_t)
            x_h = []
            for h in range(2):
                x_t = x_pool.tile([P, 2, HW], F32R, name=f"x{n}_{h}", tag=f"x{h}")
                nc.sync.dma_start(
                    x_t[:],
                    images[n][h * 256 : (h + 1) * 256].rearrange(
                        "(kd p) f -> p kd f", p=P
                    ),
                )
                x_h.append(x_t)

            if first:
                nc.gpsimd.memset(ones_f32[:], 1.0)
                nc.vector.tensor_copy(ones[:], ones_f32[:])
                first = False

            # ---- E = exp(attsT) in [d, c] layout (elementwise) ----
            e_t = e_pool.tile([P, KD, C], F32R, name=f"e{n}", tag="e")
            for h in range(2):
                nc.scalar.activation(
                    e_t[:, h * 2 : (h + 1) * 2],
                    a_h[h][:],
                    AF.Exp,
                    bias=0.0,
                    scale=1.0,
                )

            # ---- denominators, replicated to all partitions:
            #      R[p, c] = sum_d E[d, c]  (ones.T @ E, accumulated over kd)
            s_ps = sm_psum.tile([P, C], F32, name=f"s{n}", tag="s", space="PSUM")
            for kd in range(KD):
                nc.tensor.matmul(
                    s_ps[:],
                    lhsT=ones[:],
                    rhs=e_t[:, kd],
                    start=(kd == 0),
                    stop=(kd == KD - 1),
                )
            rinv = st_pool.tile([P, C], F32, name=f"rinv{n}", tag="rinv")
            nc.vector.reciprocal(rinv[:], s_ps[:])

            # normalize E columns: E[:, kd, c] *= 1/s[c]  (W rows now sum to 1)
            for kd in range(KD):
                nc.vector.tensor_mul(e_t[:, kd], e_t[:, kd], rinv[:])

            # ---- matmuls + scaled eviction ----
            o_h = [
                o_pool.tile([P, 2, HW], F32, name=f"o{n}_{h}", tag=f"o{h}")
                for h in range(2)
            ]
            for kc in range(KC):
                ps = mm_psum.tile(
                    [P, HW], F32, name=f"ps{n}_{kc}", tag="ps", space="PSUM"
                )
                for kd in range(KD):
                    for ht in range(NHT):
                        nc.tensor.matmul(
                            ps[:, ht * NT : (ht + 1) * NT],
                            lhsT=e_t[:, kd, kc * P : (kc + 1) * P],
                            rhs=x_h[kd // 2][:, kd % 2, ht * NT : (ht + 1) * NT],
                            start=(kd == 0),
                            stop=(kd == KD - 1),
                        )
                o_dst = o_h[kc // 2][:, kc % 2]
                if kc % 2 == 0:
                    nc.scalar.copy(o_dst, ps[:])
                else:
                    nc.vector.tensor_copy(o_dst, ps[:])

            for h in range(2):
                nc.gpsimd.dma_start(
                    out[n][h * 256 : (h + 1) * 256].rearrange(
                        "(kc p) f -> p kc f", p=P
                    ),
                    o_h[h][:],
                )

    nc.compile()
    return nc


_NC_CACHE = None


def _get_nc():
    global _NC_CACHE
    if _NC_CACHE is None:
        _NC_CACHE = build_nc()
    return _NC_CACHE


def run(in_maps, **kwargs):
    """Run the SPMD kernel on cores 0..7. in_maps: one dict per core."""
    nc = _get_nc()
    return run_bass_kernel_spmd(nc, in_maps, core_ids=list(range(NCORES)), **kwargs)


def make_in_maps(images: np.ndarray, atts: np.ndarray):
    images = np.ascontiguousarray(np.asarray(images, dtype=np.float32))
    atts = np.asarray(atts, dtype=np.float32)
    assert images.shape == (N, C, H, W), images.shape
    assert atts.shape == (N, C, C), atts.shape
    img_s = images.reshape(NCORES, NPC, C, HW)
    # per-sample transpose: attsT[n] = atts[n].T  (layout [d, c])
    attsT = np.ascontiguousarray(atts.transpose(0, 2, 1)).reshape(
        NCORES, NPC, C, C
    )
    return [
        {"images": np.ascontiguousarray(img_s[i]), "attsT": attsT[i]}
        for i in range(NCORES)
    ]


def kernel(images: np.ndarray, atts: np.ndarray) -> np.ndarray:
    in_maps = make_in_maps(images, atts)
    res = run(in_maps)
    outs = [res.results[i]["out"] for i in range(NCORES)]
    full = np.concatenate(outs, axis=0).reshape(N, C, H, W)
    return full.astype(np.float32)
